# revision 26
# baseline (speedup 1.0000x reference)
"""GatedAttentionUnit (B=4, N=4096, H=1024, I=2048, DK=128) on 8 trn2 cores.

Sharding: core c -> (batch b = c//2, intermediate-half h = c%2). Each core
stages a single packed input buffer (its batch's full hidden_states, and the
Wv/Wu/Wo columns of its own half of the intermediate dim I) and computes
v/u/attention over ALL 4096 rows but only its 1024 i-columns, producing a
partial output; the host sums each pair's partials (exact — the final matmul
contracts over i). k and the softmax denominators are cheap and computed
redundantly on both cores.

v and exp(scores) are stored in SBUF as fp8e4 (no DRAM round trip); the
dominant A@V matmul, the v/z projections, and the softmax-denominator matmul
run as fp8 DoubleRow (256-deep contraction per pass, ~2x bf16 equal-flops
rate). The u projection, scores, and the output matmul stay bf16 for
accuracy (u multiplies the attention output directly, so its quantization
noise is not softmax-averaged). The u projection is computed just-in-time
per 512-row query block inside the attention loop, keeping the PE stream
continuous; full hidden_states stay resident in SBUF instead of u. Softmax
denominators are transposed to per-partition layout with tiny PE transpose
matmuls instead of a DRAM round trip.
"""
import sys

sys.path.insert(0, '/opt/trn_rl_repo')

import numpy as np
import ml_dtypes

import jax
from jax.sharding import Mesh, NamedSharding, PartitionSpec
from jax.experimental.shard_map import shard_map

import concourse.bass as bass
import concourse.mybir as mybir
import concourse.tile as tile
from concourse import bass2jax
from concourse.vector_clock import ScopedClock

BF16 = mybir.dt.bfloat16
F8 = mybir.dt.float8e4
F32 = mybir.dt.float32
AF = mybir.ActivationFunctionType
DR = mybir.MatmulPerfMode.DoubleRow

B, N, H, I, DK = 4, 4096, 1024, 2048, 128
HC = H // 128         # 8 contraction chunks
NT = N // 128         # 32 kv row tiles (global)
NMB = N // 512        # 8 query blocks (all rows; i-half split)
IH = I // 2           # own half of the intermediate dim
ITC = IH // 128       # 8 i-tiles of the own half
LOG512 = float(np.log(512.0))

# ---- packed input layout (elements, bf16) ---------------------------------
OFF_HS = 0                          # hsT full [H, N] row-major
OFF_QT = OFF_HS + H * N             # qT [128, N] (all rows)
OFF_TC = OFF_QT + DK * N            # TCc full [128, N]
OFF_TS = OFF_TC + DK * N            # TSc full [128, N]
OFF_EB = OFF_TS + DK * N            # ebias [128, 32]
OFF_WU = OFF_EB + 128 * NT          # Wu own i-half [H, IH]
OFF_WO = OFF_WU + H * IH            # Wo own i-half [IH, H]
TOTAL = OFF_WO + IH * H

# ---- packed fp8 input layout (elements) -----------------------------------
# v/z-projection weights, prescaled by 16 on the host so their 0.02-sigma
# values stay in fp8e4's normal range; the silu activations compensate with
# scale=1/16.
OFF8_WV = 0                         # 16*Wv own i-half [H, IH]
OFF8_WZ = OFF8_WV + H * IH          # 16*Wzp [H, DK]
TOTAL8 = OFF8_WZ + H * DK

# ---------------------------------------------------------------------------
# Workarounds for this container's walrus build: at most ONE sync-wait per
# instruction; split extras onto same-engine NOPs (incl. the tail drain).
# ---------------------------------------------------------------------------


def _split_excess_waits(nc, max_waits=1):
    fn = nc.m.functions[0]
    for bb in fn.blocks:
        out = []
        changed = False
        for inst in bb.instructions:
            si = inst.sync_info
            if si is not None and si.on_wait and len(si.on_wait) > max_waits:
                waits = list(si.on_wait)
                extra, keep = waits[:-max_waits], waits[-max_waits:]
                for i in range(0, len(extra), max_waits):
                    nop = mybir.InstNoOp(
                        name=nc.get_next_instruction_name(),
                        sync_info=mybir.SyncInfo(
                            on_wait=extra[i:i + max_waits], on_update=[]),
                        bass_nofuse=True,
                        engine=inst.engine,
                    )
                    out.append(nop)
                si.on_wait = keep
                changed = True
            out.append(inst)
        if changed:
            bb.instructions = out


class CompatTileContext(tile.TileContext):
    def _drain_and_barrier(self, tick_clock, wait_clock):
        carrier = self.nc.sync.nop(nofuse=True, hint="drain_waits")
        wait_clock.add_sem_waits(
            carrier.ins, ScopedClock({None: tick_clock.global_clock}))
        si = carrier.ins.sync_info
        waits = list(si.on_wait) if si and si.on_wait else []
        if si:
            si.on_wait = waits[:1]
        for w in waits[1:]:
            extra = self.nc.sync.nop(nofuse=True, hint="drain_waits")
            extra.ins.sync_info = mybir.SyncInfo(on_wait=[w], on_update=[])
        self.nc.sync.drain()
        self.nc.all_engine_barrier()
        assert self.sems is not None
        popped = self.nc._tile_sem_poison_stack.pop()
        assert popped is self._sem_poison
        self.nc.clear_and_free_semaphores(list(self.sems.allocated().values()))
        self.nc.all_engine_barrier()

    def __exit__(self, exc_type, exc_value, traceback):
        r = super().__exit__(exc_type, exc_value, traceback)
        if exc_type is None:
            _split_excess_waits(self.nc)
        return r


# ---------------------------------------------------------------------------
# Device program (shared SPMD across the 8 cores; all per-core variation is
# carried by the input data).
# ---------------------------------------------------------------------------

def build_program():
    nc = bass.Bass('TRN2', target_bir_lowering=False, num_devices=8)

    inp = nc.declare_dram_parameter('inp', [TOTAL], BF16, isOutput=False)
    inp8 = nc.declare_dram_parameter('inp8', [TOTAL8], F8, isOutput=False)
    o_out = nc.declare_dram_parameter('o', [N, H], BF16, isOutput=True)

    hsT_re = inp[OFF_HS:OFF_HS + H * N].rearrange('(c p n) -> p c n', p=128, c=HC)
    wu_view = inp[OFF_WU:OFF_WU + H * IH].rearrange('(c p n) -> p c n', p=128, c=HC)

    with CompatTileContext(nc) as tc:
        with tc.tile_pool(name='pers', bufs=1) as pers:

            # ---- persistent tiles -----------------------------------------
            kT_t = pers.tile([128, N], BF16, tag='kT')
            eb_bf = pers.tile([128, NT], BF16, tag='ebb')
            nc.sync.dma_start(
                out=eb_bf[:],
                in_=inp[OFF_EB:OFF_EB + 128 * NT].rearrange('(p n) -> p n', p=128))
            eb_t = pers.tile([128, NT], F32, tag='eb')
            nc.vector.tensor_copy(eb_t[:], eb_bf[:])
            ones_f = pers.tile([128, 2, 16], F32, tag='onesf')
            nc.vector.memset(ones_f[:], 1.0)
            ones8 = pers.tile([128, 2, 16], F8, tag='ones8')
            nc.vector.tensor_copy(ones8[:], ones_f[:])
            id1 = pers.tile([1, 1], F32, tag='id1')
            nc.vector.memset(id1[:], 1.0)
            v8_t = pers.tile([128, NT, IH], F8, tag='v8')    # v [n, i] fp8

            NQ = 4
            QN = N // NQ    # 1024 rows per hidden_states quarter
            with tc.tile_pool(name='phA', bufs=1) as phA, \
                 tc.tile_pool(name='hsq', bufs=2) as hsq, \
                 tc.tile_pool(name='hs8p', bufs=2) as hs8p, \
                 tc.tile_pool(name='zk', bufs=3) as zkp, \
                 tc.tile_pool(name='psA', bufs=8, space='PSUM') as psA:

                Wv8_t = phA.tile([128, HC, IH], F8, tag='Wv8')
                nc.scalar.dma_start(
                    out=Wv8_t[:],
                    in_=inp8[OFF8_WV:OFF8_WV + H * IH].rearrange(
                        '(c p n) -> p c n', p=128, c=HC))
                Wz8_t = phA.tile([128, HC, DK], F8, tag='Wz8')
                nc.sync.dma_start(
                    out=Wz8_t[:],
                    in_=inp8[OFF8_WZ:OFF8_WZ + H * DK].rearrange(
                        '(c p n) -> p c n', p=128, c=HC))
                TC_t = phA.tile([128, N], BF16, tag='TC')
                nc.sync.dma_start(
                    out=TC_t[:],
                    in_=inp[OFF_TC:OFF_TC + DK * N].rearrange('(p n) -> p n', p=128))
                TS_t = phA.tile([128, N], BF16, tag='TS')
                nc.sync.dma_start(
                    out=TS_t[:],
                    in_=inp[OFF_TS:OFF_TS + DK * N].rearrange('(p n) -> p n', p=128))

                # ---- v projection + z->k over all rows, fp8 DoubleRow -----
                for q in range(NQ):
                    qsl = slice(q * QN, (q + 1) * QN)
                    hsQ = hsq.tile([128, HC, QN], BF16, tag='hsQ', name=f'hsQ{q}')
                    hs8 = hs8p.tile([128, HC, QN], F8, tag='hs8', name=f'hs8{q}')
                    for hq in range(2):
                        ssl = slice(hq * (QN // 2), (hq + 1) * (QN // 2))
                        gsl = slice(q * QN + hq * (QN // 2),
                                    q * QN + (hq + 1) * (QN // 2))
                        nc.sync.dma_start(out=hsQ[:, :, ssl], in_=hsT_re[:, :, gsl])
                        nc.vector.tensor_copy(hs8[:, :, ssl], hsQ[:, :, ssl])

                    for lt in range(QN // 128):
                        nt = q * (QN // 128) + lt
                        csl = slice(nt * 128, (nt + 1) * 128)
                        lsl = slice(lt * 128, (lt + 1) * 128)
                        # z chunk
                        pz = psA.tile([128, 128], F32, tag='ps', name=f'pz{nt}')
                        for hc in range(HC // 2):
                            nc.tensor.matmul(pz[:], Wz8_t[:, 2 * hc:2 * hc + 2, :],
                                             hs8[:, 2 * hc:2 * hc + 2, lsl],
                                             start=(hc == 0), stop=(hc == HC // 2 - 1),
                                             perf_mode=DR)
                        zT = zkp.tile([128, 128], BF16, tag='zT', name=f'zT{nt}')
                        nc.scalar.activation(zT[:], pz[:], AF.Silu, scale=1.0 / 16.0)
                        zsw = zkp.tile([128, 128], BF16, tag='zsw', name=f'zsw{nt}')
                        nc.vector.tensor_copy(zsw[0:64, :], zT[64:128, :])
                        nc.vector.tensor_copy(zsw[64:128, :], zT[0:64, :])
                        t1 = zkp.tile([128, 128], BF16, tag='t1', name=f't1_{nt}')
                        nc.vector.tensor_mul(t1[:], zT[:], TC_t[:, csl])
                        t2 = zkp.tile([128, 128], BF16, tag='t2', name=f't2_{nt}')
                        nc.vector.tensor_mul(t2[:], zsw[:], TS_t[:, csl])
                        nc.vector.tensor_sub(kT_t[0:64, csl], t1[0:64, :], t2[0:64, :])
                        nc.vector.tensor_add(kT_t[64:128, csl], t1[64:128, :], t2[64:128, :])
                        # v row-tile -> persistent fp8
                        pv = [psA.tile([128, 512], F32, tag='ps', name=f'pv{nt}_{j}')
                              for j in range(2)]
                        for hc in range(HC // 2):
                            lhs = hs8[:, 2 * hc:2 * hc + 2, lsl]
                            for ic in range(2):
                                nc.tensor.matmul(pv[ic][:], lhs,
                                                 Wv8_t[:, 2 * hc:2 * hc + 2,
                                                       ic * 512:(ic + 1) * 512],
                                                 start=(hc == 0),
                                                 stop=(hc == HC // 2 - 1),
                                                 perf_mode=DR)
                        for ic in range(2):
                            nc.scalar.activation(v8_t[:, nt, ic * 512:(ic + 1) * 512],
                                                 pv[ic][:], AF.Silu, scale=1.0 / 16.0)

            # ---- attention + u-projection + output, per 512-row block -----
            with tc.tile_pool(name='ph2', bufs=1) as ph2, \
                 tc.tile_pool(name='atp', bufs=2) as atp, \
                 tc.tile_pool(name='utp', bufs=2) as utp, \
                 tc.tile_pool(name='hsb', bufs=2) as hsbp, \
                 tc.tile_pool(name='wblk', bufs=1) as wblk, \
                 tc.tile_pool(name='fin', bufs=2) as finp, \
                 tc.tile_pool(name='ps_s', bufs=4, space='PSUM') as ps_s, \
                 tc.tile_pool(name='ps_av', bufs=2, space='PSUM') as ps_av, \
                 tc.tile_pool(name='ps_f', bufs=2, space='PSUM') as ps_f:

                Wu_t = ph2.tile([128, HC, IH], BF16, tag='Wu')
                nc.scalar.dma_start(out=Wu_t[:], in_=wu_view[:, :, :])
                Wo_t = ph2.tile([128, ITC, H], BF16, tag='Wo')
                nc.scalar.dma_start(
                    out=Wo_t[:],
                    in_=inp[OFF_WO:OFF_WO + IH * H].rearrange(
                        '(c p n) -> p c n', p=128, c=ITC))
                qT_t = ph2.tile([128, N], BF16, tag='qT')
                nc.sync.dma_start(
                    out=qT_t[:],
                    in_=inp[OFF_QT:OFF_QT + DK * N].rearrange('(p n) -> p n', p=128))

                prev_at = None
                prev_ut = None
                prev_mb = -1
                for mb in range(NMB + 1):
                    msl = slice(mb * 512, (mb + 1) * 512)
                    cur_at = None
                    cur_ut = None
                    cur_w = None
                    if mb < NMB:
                        cur_at = atp.tile([128, NT, 512], F8, tag='at',
                                          name=f'at{mb}')
                        cur_ut = utp.tile([128, ITC, 512], BF16, tag='ut',
                                          name=f'ut{mb}')
                        hsb = hsbp.tile([128, HC, 512], BF16, tag='hsb',
                                        name=f'hsb{mb}')
                        nc.sync.dma_start(out=hsb[:], in_=hsT_re[:, :, msl])
                    if prev_at is not None:
                        cur_w = wblk.tile([128, ITC, 512], BF16, tag='w',
                                          name=f'w{prev_mb}')

                    # denominators for prev block first: the scalar-copy ->
                    # transpose -> reciprocal chain overlaps scores/AV below.
                    if prev_at is not None:
                        psum_s = ps_f.tile([16, 512], F32, tag='psf',
                                           name=f'psum_s{prev_mb}')
                        for t in range(NT // 2):
                            nc.tensor.matmul(psum_s[:], ones8[:],
                                             prev_at[:, 2 * t:2 * t + 2, :],
                                             start=(t == 0), stop=(t == NT // 2 - 1),
                                             perf_mode=DR)
                        sums_sb = finp.tile([1, 512], F32, tag='sums',
                                            name=f'sums{prev_mb}')
                        nc.scalar.copy(sums_sb[:], psum_s[0:1, :])

                    # interleave: scores for mb (groups of 4 kv tiles) with
                    # AV for prev block (one i-tile per group).
                    rinv = None
                    for g in range(8):
                        if mb < NMB:
                            for j in range(4):
                                nt = 4 * g + j
                                pss = ps_s.tile([128, 512], F32, tag='pss',
                                                name=f'pss{mb}_{nt}')
                                nc.tensor.matmul(pss[:],
                                                 kT_t[:, nt * 128:(nt + 1) * 128],
                                                 qT_t[:, msl], start=True, stop=True)
                                nc.scalar.activation(cur_at[:, nt, :], pss[:],
                                                     AF.Exp,
                                                     bias=eb_t[:, nt:nt + 1],
                                                     scale=1.0)
                        if prev_at is not None:
                            if g == 1:
                                # denominator transpose, after the first score
                                # group so the PE never waits on the scalar
                                # copy of psum_s.
                                rinv = finp.tile([128, 4], F32, tag='rinv',
                                                 name=f'rinv{prev_mb}')
                                for mt in range(4):
                                    ptr = ps_f.tile([128, 1], F32, tag='psf',
                                                    name=f'ptr{prev_mb}_{mt}')
                                    nc.tensor.matmul(
                                        ptr[:],
                                        sums_sb[0:1, mt * 128:(mt + 1) * 128],
                                        id1[:], is_transpose=True)
                                    nc.vector.reciprocal(rinv[:, mt:mt + 1], ptr[:])
                            it = g
                            po = ps_av.tile([128, 512], F32, tag='po',
                                            name=f'po{prev_mb}_{it}')
                            for t in range(NT // 2):
                                nc.tensor.matmul(
                                    po[:],
                                    v8_t[:, 2 * t:2 * t + 2, it * 128:(it + 1) * 128],
                                    prev_at[:, 2 * t:2 * t + 2, :],
                                    start=(t == 0), stop=(t == NT // 2 - 1),
                                    perf_mode=DR)
                            nc.vector.tensor_mul(cur_w[:, it, :], po[:],
                                                 prev_ut[:, it, :])

                    # u^T projection for THIS block (consumed next iteration).
                    if mb < NMB:
                        for it in range(ITC):
                            pu = ps_s.tile([128, 512], F32, tag='pss',
                                           name=f'pu{mb}_{it}')
                            for hc in range(HC):
                                nc.tensor.matmul(pu[:],
                                                 Wu_t[:, hc, it * 128:(it + 1) * 128],
                                                 hsb[:, hc, :],
                                                 start=(hc == 0), stop=(hc == HC - 1))
                            nc.scalar.activation(cur_ut[:, it, :], pu[:], AF.Silu)

                    # final: o[m-128, H] = sum_i w^T[:, m-tile].T @ Wo, scaled
                    if prev_at is not None:
                        for mt in range(4):
                            pf = [ps_f.tile([128, 512], F32, tag='psf',
                                            name=f'pf{prev_mb}_{mt}_{j}')
                                  for j in range(2)]
                            for it in range(ITC):
                                lhs = cur_w[:, it, mt * 128:(mt + 1) * 128]
                                for oc in range(2):
                                    nc.tensor.matmul(pf[oc][:], lhs,
                                                     Wo_t[:, it, oc * 512:(oc + 1) * 512],
                                                     start=(it == 0),
                                                     stop=(it == ITC - 1))
                            osb = finp.tile([128, H], BF16, tag='osb',
                                            name=f'osb{prev_mb}_{mt}')
                            for oc in range(2):
                                nc.scalar.activation(
                                    osb[:, oc * 512:(oc + 1) * 512], pf[oc][:],
                                    AF.Copy, bias=0.0, scale=rinv[:, mt:mt + 1])
                            row = prev_mb * 512 + mt * 128
                            nc.sync.dma_start(out=o_out[row:row + 128, :],
                                              in_=osb[:])

                    prev_at = cur_at
                    prev_ut = cur_ut
                    prev_mb = mb

    return nc


_CACHED = {}


def _prep_inputs(hidden_states, x_gcn, attention_mask, sin, cos, Wi, Wo, k_scale):
    bf = ml_dtypes.bfloat16
    f8 = ml_dtypes.float8_e4m3
    Wu = np.ascontiguousarray(Wi[:, :I]).astype(bf)
    Wz = Wi[:, 2 * I:]
    Wo_b = np.ascontiguousarray(Wo).astype(bf)
    Wv8 = (Wi[:, I:2 * I].astype(np.float32) * 16.0).astype(f8)
    Wz8_full = (np.concatenate([Wz[:, 0::2], Wz[:, 1::2]], axis=1)
                .astype(np.float32) * 16.0).astype(f8)

    sin2 = sin[0]          # [N, 64]
    cos2 = cos[0]
    kse, kso = k_scale[0::2], k_scale[1::2]
    TCc = np.concatenate([(cos2 * kse).T, (cos2 * kso).T], axis=0).astype(bf)
    TSc = np.concatenate([(sin2 * kso).T, (sin2 * kse).T], axis=0).astype(bf)

    # rotary(q) with softmax_plus scale folded in, per batch
    x1, x2 = x_gcn[..., 0::2], x_gcn[..., 1::2]
    c_, s_ = cos2[None], sin2[None]
    q_rot = np.concatenate([x1 * c_ - x2 * s_, x2 * c_ + x1 * s_], axis=-1)

    flats, flats8 = [], []
    for core in range(8):
        b, h = core // 2, core % 2
        l = float(attention_mask[b].sum())
        sc = np.log(l) / LOG512 / np.sqrt(DK)
        ebias = np.where(attention_mask[b] == 0, -30.0, 0.0)
        flat = np.empty(TOTAL, bf)
        flat[OFF_HS:OFF_HS + H * N] = np.ascontiguousarray(
            hidden_states[b].T).astype(bf).ravel()
        flat[OFF_QT:OFF_QT + DK * N] = np.ascontiguousarray(
            (q_rot[b] * sc).T).astype(bf).ravel()
        flat[OFF_TC:OFF_TC + DK * N] = TCc.ravel()
        flat[OFF_TS:OFF_TS + DK * N] = TSc.ravel()
        flat[OFF_EB:OFF_EB + 128 * NT] = np.ascontiguousarray(
            ebias.reshape(NT, 128).T).astype(bf).ravel()
        i0 = h * IH
        flat[OFF_WU:OFF_WU + H * IH] = np.ascontiguousarray(Wu[:, i0:i0 + IH]).ravel()
        flat[OFF_WO:OFF_WO + IH * H] = np.ascontiguousarray(Wo_b[i0:i0 + IH, :]).ravel()
        flats.append(flat)
        flat8 = np.empty(TOTAL8, f8)
        flat8[OFF8_WV:OFF8_WV + H * IH] = np.ascontiguousarray(
            Wv8[:, i0:i0 + IH]).ravel()
        flat8[OFF8_WZ:OFF8_WZ + H * DK] = Wz8_full.ravel()
        flats8.append(flat8)
    return flats, flats8


def _get_runner():
    if 'runner' in _CACHED:
        return _CACHED['runner']
    nc = build_program()
    bass2jax.install_neuronx_cc_hook()
    pn = nc.partition_id_tensor.name if nc.partition_id_tensor else None
    in_names, out_names, out_avals = [], [], []
    for alloc in nc.m.functions[0].allocations:
        if not isinstance(alloc, mybir.MemoryLocationSet):
            continue
        name = alloc.memorylocations[0].name
        if alloc.kind == 'ExternalInput':
            if name != pn:
                in_names.append(name)
        elif alloc.kind == 'ExternalOutput':
            out_names.append(name)
            shape = tuple(alloc.tensor_shape)
            dtype = mybir.dt.np(alloc.dtype)
            out_avals.append(jax.core.ShapedArray(shape, dtype))
    n_params = len(in_names)
    if pn is not None:
        in_names.append(pn)

    def _body(*args):
        ops = list(args)
        if pn is not None:
            ops.append(bass2jax.partition_id_tensor())
        return tuple(bass2jax._bass_exec_p.bind(
            *ops, out_avals=tuple(out_avals), in_names=tuple(in_names),
            out_names=tuple(out_names), lowering_input_output_aliases=(),
            sim_require_finite=True, sim_require_nnan=True, nc=nc))

    mesh = Mesh(np.asarray(jax.devices()[:8]), ('core',))
    sharding = NamedSharding(mesh, PartitionSpec('core'))
    sized = {'inp': (TOTAL, ml_dtypes.bfloat16),
             'inp8': (TOTAL8, ml_dtypes.float8_e4m3)}
    in_specs = tuple(
        jax.ShapeDtypeStruct((8 * sized[nm][0],), sized[nm][1],
                             sharding=sharding)
        for nm in in_names[:n_params])
    # AOT-compile with bass_effect suppressed so per-call dispatch takes
    # JAX's C++ fast path.
    sharded = bass2jax.fast_dispatch_compile(
        lambda: jax.jit(
            shard_map(_body, mesh=mesh,
                      in_specs=(PartitionSpec('core'),) * n_params,
                      out_specs=(PartitionSpec('core'),) * len(out_names),
                      check_rep=False),
            keep_unused=True).lower(*in_specs).compile())

    def put(arr):
        return jax.device_put(arr, sharding)

    _CACHED['runner'] = (nc, sharded, put, tuple(in_names[:n_params]))
    return _CACHED['runner']


def kernel(hidden_states, x_gcn, attention_mask, sin, cos, Wi, Wo, k_scale):
    _, sharded, put, in_names = _get_runner()
    flats, flats8 = _prep_inputs(np.asarray(hidden_states, np.float32),
                                 np.asarray(x_gcn, np.float32),
                                 np.asarray(attention_mask),
                                 np.asarray(sin, np.float32),
                                 np.asarray(cos, np.float32),
                                 np.asarray(Wi, np.float32),
                                 np.asarray(Wo, np.float32),
                                 np.asarray(k_scale, np.float32))
    cat = {'inp': np.concatenate(flats, axis=0),
           'inp8': np.concatenate(flats8, axis=0)}
    args = tuple(put(cat[nm]) for nm in in_names)
    res = np.asarray(sharded(*args)[0]).reshape(8, N, H).astype(np.float32)
    out = np.empty((B, N, H), np.float32)
    for b in range(B):
        out[b] = res[2 * b] + res[2 * b + 1]
    return out


# revision 28
# speedup vs baseline: 1.0257x; 1.0257x over previous
"""GatedAttentionUnit (B=4, N=4096, H=1024, I=2048, DK=128) on 8 trn2 cores.

Sharding: core c -> (batch b = c//2, intermediate-half h = c%2). Each core
stages a single packed input buffer (its batch's full hidden_states, and the
Wv/Wu/Wo columns of its own half of the intermediate dim I) and computes
v/u/attention over ALL 4096 rows but only its 1024 i-columns, producing a
partial output; the host sums each pair's partials (exact — the final matmul
contracts over i). k and the softmax denominators are cheap and computed
redundantly on both cores.

v and exp(scores) are stored in SBUF as fp8e4 (no DRAM round trip); the
dominant A@V matmul, the v/z projections, and the softmax-denominator matmul
run as fp8 DoubleRow (256-deep contraction per pass, ~2x bf16 equal-flops
rate). The u projection, scores, and the output matmul stay bf16 for
accuracy (u multiplies the attention output directly, so its quantization
noise is not softmax-averaged). The u projection is computed just-in-time
per 512-row query block inside the attention loop, keeping the PE stream
continuous; full hidden_states stay resident in SBUF instead of u. Softmax
denominators are transposed to per-partition layout with tiny PE transpose
matmuls instead of a DRAM round trip.
"""
import sys

sys.path.insert(0, '/opt/trn_rl_repo')

import numpy as np
import ml_dtypes

import jax
from jax.sharding import Mesh, NamedSharding, PartitionSpec
from jax.experimental.shard_map import shard_map

import concourse.bass as bass
import concourse.mybir as mybir
import concourse.tile as tile
from concourse import bass2jax
from concourse.vector_clock import ScopedClock

BF16 = mybir.dt.bfloat16
F8 = mybir.dt.float8e4
F32 = mybir.dt.float32
AF = mybir.ActivationFunctionType
DR = mybir.MatmulPerfMode.DoubleRow

B, N, H, I, DK = 4, 4096, 1024, 2048, 128
HC = H // 128         # 8 contraction chunks
NT = N // 128         # 32 kv row tiles (global)
NMB = N // 512        # 8 query blocks (all rows; i-half split)
IH = I // 2           # own half of the intermediate dim
ITC = IH // 128       # 8 i-tiles of the own half
LOG512 = float(np.log(512.0))

# ---- packed input layout (elements, bf16) ---------------------------------
OFF_HS = 0                          # hsT full [H, N] row-major
OFF_QT = OFF_HS + H * N             # qT [128, N] (all rows)
OFF_TC = OFF_QT + DK * N            # TCc full [128, N]
OFF_TS = OFF_TC + DK * N            # TSc full [128, N]
OFF_EB = OFF_TS + DK * N            # ebias [128, 32]
OFF_WU = OFF_EB + 128 * NT          # Wu own i-half [H, IH]
OFF_WO = OFF_WU + H * IH            # Wo own i-half [IH, H]
TOTAL = OFF_WO + IH * H

# ---- packed fp8 input layout (elements) -----------------------------------
# v/z-projection weights, prescaled by 16 on the host so their 0.02-sigma
# values stay in fp8e4's normal range; the silu activations compensate with
# scale=1/16.
OFF8_WV = 0                         # 16*Wv own i-half [H, IH]
OFF8_WZ = OFF8_WV + H * IH          # 16*Wzp [H, DK]
TOTAL8 = OFF8_WZ + H * DK

# ---------------------------------------------------------------------------
# Workarounds for this container's walrus build: at most ONE sync-wait per
# instruction; split extras onto same-engine NOPs (incl. the tail drain).
# ---------------------------------------------------------------------------


def _split_excess_waits(nc, max_waits=1):
    fn = nc.m.functions[0]
    for bb in fn.blocks:
        out = []
        changed = False
        for inst in bb.instructions:
            si = inst.sync_info
            if si is not None and si.on_wait and len(si.on_wait) > max_waits:
                waits = list(si.on_wait)
                extra, keep = waits[:-max_waits], waits[-max_waits:]
                for i in range(0, len(extra), max_waits):
                    nop = mybir.InstNoOp(
                        name=nc.get_next_instruction_name(),
                        sync_info=mybir.SyncInfo(
                            on_wait=extra[i:i + max_waits], on_update=[]),
                        bass_nofuse=True,
                        engine=inst.engine,
                    )
                    out.append(nop)
                si.on_wait = keep
                changed = True
            out.append(inst)
        if changed:
            bb.instructions = out


class CompatTileContext(tile.TileContext):
    def _drain_and_barrier(self, tick_clock, wait_clock):
        carrier = self.nc.sync.nop(nofuse=True, hint="drain_waits")
        wait_clock.add_sem_waits(
            carrier.ins, ScopedClock({None: tick_clock.global_clock}))
        si = carrier.ins.sync_info
        waits = list(si.on_wait) if si and si.on_wait else []
        if si:
            si.on_wait = waits[:1]
        for w in waits[1:]:
            extra = self.nc.sync.nop(nofuse=True, hint="drain_waits")
            extra.ins.sync_info = mybir.SyncInfo(on_wait=[w], on_update=[])
        self.nc.sync.drain()
        self.nc.all_engine_barrier()
        assert self.sems is not None
        popped = self.nc._tile_sem_poison_stack.pop()
        assert popped is self._sem_poison
        self.nc.clear_and_free_semaphores(list(self.sems.allocated().values()))
        self.nc.all_engine_barrier()

    def __exit__(self, exc_type, exc_value, traceback):
        r = super().__exit__(exc_type, exc_value, traceback)
        if exc_type is None:
            _split_excess_waits(self.nc)
        return r


# ---------------------------------------------------------------------------
# Device program (shared SPMD across the 8 cores; all per-core variation is
# carried by the input data).
# ---------------------------------------------------------------------------

def build_program():
    nc = bass.Bass('TRN2', target_bir_lowering=False, num_devices=8)

    inp = nc.declare_dram_parameter('inp', [TOTAL], BF16, isOutput=False)
    inp8 = nc.declare_dram_parameter('inp8', [TOTAL8], F8, isOutput=False)
    o_out = nc.declare_dram_parameter('o', [N, H], BF16, isOutput=True)

    hsT_re = inp[OFF_HS:OFF_HS + H * N].rearrange('(c p n) -> p c n', p=128, c=HC)
    wu_view = inp[OFF_WU:OFF_WU + H * IH].rearrange('(c p n) -> p c n', p=128, c=HC)

    with CompatTileContext(nc) as tc:
        with tc.tile_pool(name='pers', bufs=1) as pers:

            # ---- persistent tiles -----------------------------------------
            kT_t = pers.tile([128, N], BF16, tag='kT')
            eb_bf = pers.tile([128, NT], BF16, tag='ebb')
            nc.sync.dma_start(
                out=eb_bf[:],
                in_=inp[OFF_EB:OFF_EB + 128 * NT].rearrange('(p n) -> p n', p=128))
            eb_t = pers.tile([128, NT], F32, tag='eb')
            nc.vector.tensor_copy(eb_t[:], eb_bf[:])
            ones_f = pers.tile([128, 2, 16], F32, tag='onesf')
            nc.vector.memset(ones_f[:], 1.0)
            ones8 = pers.tile([128, 2, 16], F8, tag='ones8')
            nc.vector.tensor_copy(ones8[:], ones_f[:])
            id1 = pers.tile([1, 1], F32, tag='id1')
            nc.vector.memset(id1[:], 1.0)
            v8_t = pers.tile([128, NT, IH], F8, tag='v8')    # v [n, i] fp8

            NQ = 4
            QN = N // NQ    # 1024 rows per hidden_states quarter
            with tc.tile_pool(name='phA', bufs=1) as phA, \
                 tc.tile_pool(name='hsq', bufs=2) as hsq, \
                 tc.tile_pool(name='hs8p', bufs=2) as hs8p, \
                 tc.tile_pool(name='zk', bufs=3) as zkp, \
                 tc.tile_pool(name='psA', bufs=8, space='PSUM') as psA:

                Wv8_t = phA.tile([128, HC, IH], F8, tag='Wv8')
                nc.scalar.dma_start(
                    out=Wv8_t[:],
                    in_=inp8[OFF8_WV:OFF8_WV + H * IH].rearrange(
                        '(c p n) -> p c n', p=128, c=HC))
                Wz8_t = phA.tile([128, HC, DK], F8, tag='Wz8')
                nc.sync.dma_start(
                    out=Wz8_t[:],
                    in_=inp8[OFF8_WZ:OFF8_WZ + H * DK].rearrange(
                        '(c p n) -> p c n', p=128, c=HC))
                TC_t = phA.tile([128, N], BF16, tag='TC')
                nc.sync.dma_start(
                    out=TC_t[:],
                    in_=inp[OFF_TC:OFF_TC + DK * N].rearrange('(p n) -> p n', p=128))
                TS_t = phA.tile([128, N], BF16, tag='TS')
                nc.sync.dma_start(
                    out=TS_t[:],
                    in_=inp[OFF_TS:OFF_TS + DK * N].rearrange('(p n) -> p n', p=128))

                # ---- v projection + z->k over all rows, fp8 DoubleRow -----
                for q in range(NQ):
                    qsl = slice(q * QN, (q + 1) * QN)
                    hsQ = hsq.tile([128, HC, QN], BF16, tag='hsQ', name=f'hsQ{q}')
                    hs8 = hs8p.tile([128, HC, QN], F8, tag='hs8', name=f'hs8{q}')
                    for hq in range(2):
                        ssl = slice(hq * (QN // 2), (hq + 1) * (QN // 2))
                        gsl = slice(q * QN + hq * (QN // 2),
                                    q * QN + (hq + 1) * (QN // 2))
                        nc.sync.dma_start(out=hsQ[:, :, ssl], in_=hsT_re[:, :, gsl])
                        nc.vector.tensor_copy(hs8[:, :, ssl], hsQ[:, :, ssl])

                    for lt in range(QN // 128):
                        nt = q * (QN // 128) + lt
                        csl = slice(nt * 128, (nt + 1) * 128)
                        lsl = slice(lt * 128, (lt + 1) * 128)
                        # z chunk
                        pz = psA.tile([128, 128], F32, tag='ps', name=f'pz{nt}')
                        for hc in range(HC // 2):
                            nc.tensor.matmul(pz[:], Wz8_t[:, 2 * hc:2 * hc + 2, :],
                                             hs8[:, 2 * hc:2 * hc + 2, lsl],
                                             start=(hc == 0), stop=(hc == HC // 2 - 1),
                                             perf_mode=DR)
                        zT = zkp.tile([128, 128], BF16, tag='zT', name=f'zT{nt}')
                        nc.scalar.activation(zT[:], pz[:], AF.Silu, scale=1.0 / 16.0)
                        zsw = zkp.tile([128, 128], BF16, tag='zsw', name=f'zsw{nt}')
                        nc.vector.tensor_copy(zsw[0:64, :], zT[64:128, :])
                        nc.vector.tensor_copy(zsw[64:128, :], zT[0:64, :])
                        t1 = zkp.tile([128, 128], BF16, tag='t1', name=f't1_{nt}')
                        nc.vector.tensor_mul(t1[:], zT[:], TC_t[:, csl])
                        t2 = zkp.tile([128, 128], BF16, tag='t2', name=f't2_{nt}')
                        nc.vector.tensor_mul(t2[:], zsw[:], TS_t[:, csl])
                        nc.vector.tensor_sub(kT_t[0:64, csl], t1[0:64, :], t2[0:64, :])
                        nc.vector.tensor_add(kT_t[64:128, csl], t1[64:128, :], t2[64:128, :])
                        # v row-tile -> persistent fp8
                        pv = [psA.tile([128, 512], F32, tag='ps', name=f'pv{nt}_{j}')
                              for j in range(2)]
                        for hc in range(HC // 2):
                            lhs = hs8[:, 2 * hc:2 * hc + 2, lsl]
                            for ic in range(2):
                                nc.tensor.matmul(pv[ic][:], lhs,
                                                 Wv8_t[:, 2 * hc:2 * hc + 2,
                                                       ic * 512:(ic + 1) * 512],
                                                 start=(hc == 0),
                                                 stop=(hc == HC // 2 - 1),
                                                 perf_mode=DR)
                        for ic in range(2):
                            nc.scalar.activation(v8_t[:, nt, ic * 512:(ic + 1) * 512],
                                                 pv[ic][:], AF.Silu, scale=1.0 / 16.0)

            # ---- attention + u-projection + output, per 512-row block -----
            with tc.tile_pool(name='ph2', bufs=1) as ph2, \
                 tc.tile_pool(name='atp', bufs=2) as atp, \
                 tc.tile_pool(name='utp', bufs=2) as utp, \
                 tc.tile_pool(name='hsb', bufs=2) as hsbp, \
                 tc.tile_pool(name='wblk', bufs=1) as wblk, \
                 tc.tile_pool(name='fin', bufs=2) as finp, \
                 tc.tile_pool(name='ps_s', bufs=4, space='PSUM') as ps_s, \
                 tc.tile_pool(name='ps_av', bufs=2, space='PSUM') as ps_av, \
                 tc.tile_pool(name='ps_f', bufs=2, space='PSUM') as ps_f:

                Wu_t = ph2.tile([128, HC, IH], BF16, tag='Wu')
                nc.scalar.dma_start(out=Wu_t[:], in_=wu_view[:, :, :])
                Wo_t = ph2.tile([128, ITC, H], BF16, tag='Wo')
                nc.scalar.dma_start(
                    out=Wo_t[:],
                    in_=inp[OFF_WO:OFF_WO + IH * H].rearrange(
                        '(c p n) -> p c n', p=128, c=ITC))
                qT_t = ph2.tile([128, N], BF16, tag='qT')
                nc.sync.dma_start(
                    out=qT_t[:],
                    in_=inp[OFF_QT:OFF_QT + DK * N].rearrange('(p n) -> p n', p=128))

                # last 512-row block is split into two 256-row blocks so the
                # un-overlapped flush tail (AV+final of the final block with
                # no next block's scores to interleave) is half as long.
                BL = [(i * 512, 512) for i in range(NMB - 1)] + \
                     [((NMB - 1) * 512, 256), ((NMB - 1) * 512 + 256, 256)]

                prev_at = None
                prev_ut = None
                prev = None
                for idx in range(len(BL) + 1):
                    cur = BL[idx] if idx < len(BL) else None
                    cur_at = None
                    cur_ut = None
                    cur_w = None
                    if cur is not None:
                        moff, bw = cur
                        msl = slice(moff, moff + bw)
                        sfx = '' if bw == 512 else '2'
                        cur_at = atp.tile([128, NT, bw], F8, tag='at' + sfx,
                                          name=f'at{idx}')
                        cur_ut = utp.tile([128, ITC, bw], BF16, tag='ut' + sfx,
                                          name=f'ut{idx}')
                        hsb = hsbp.tile([128, HC, bw], BF16, tag='hsb' + sfx,
                                        name=f'hsb{idx}')
                        nc.sync.dma_start(out=hsb[:], in_=hsT_re[:, :, msl])
                    if prev_at is not None:
                        pmoff, pbw = prev
                        psfx = '' if pbw == 512 else '2'
                        pmt = pbw // 128
                        cur_w = wblk.tile([128, ITC, pbw], BF16, tag='w' + psfx,
                                          name=f'w{idx - 1}')

                    # denominators for prev block first: the scalar-copy ->
                    # transpose -> reciprocal chain overlaps scores/AV below.
                    if prev_at is not None:
                        psum_s = ps_f.tile([16, 512], F32, tag='psf',
                                           name=f'psum_s{idx - 1}')
                        for t in range(NT // 2):
                            nc.tensor.matmul(psum_s[:, 0:pbw], ones8[:],
                                             prev_at[:, 2 * t:2 * t + 2, :],
                                             start=(t == 0), stop=(t == NT // 2 - 1),
                                             perf_mode=DR)
                        sums_sb = finp.tile([1, 512], F32, tag='sums',
                                            name=f'sums{idx - 1}')
                        nc.scalar.copy(sums_sb[0:1, 0:pbw], psum_s[0:1, 0:pbw])

                    # interleave: scores for cur (groups of 4 kv tiles) with
                    # AV for prev block (one i-tile per group).
                    rinv = None
                    for g in range(8):
                        if cur is not None:
                            for j in range(4):
                                nt = 4 * g + j
                                pss = ps_s.tile([128, 512], F32, tag='pss',
                                                name=f'pss{idx}_{nt}')
                                nc.tensor.matmul(pss[:, 0:bw],
                                                 kT_t[:, nt * 128:(nt + 1) * 128],
                                                 qT_t[:, msl], start=True, stop=True)
                                nc.scalar.activation(cur_at[:, nt, :],
                                                     pss[:, 0:bw], AF.Exp,
                                                     bias=eb_t[:, nt:nt + 1],
                                                     scale=1.0)
                        if prev_at is not None:
                            if g == 1:
                                # denominator transpose, after the first score
                                # group so the PE never waits on the scalar
                                # copy of psum_s.
                                rinv = finp.tile([128, 4], F32, tag='rinv',
                                                 name=f'rinv{idx - 1}')
                                for mt in range(pmt):
                                    ptr = ps_f.tile([128, 1], F32, tag='psf',
                                                    name=f'ptr{idx - 1}_{mt}')
                                    nc.tensor.matmul(
                                        ptr[:],
                                        sums_sb[0:1, mt * 128:(mt + 1) * 128],
                                        id1[:], is_transpose=True)
                                    nc.vector.reciprocal(rinv[:, mt:mt + 1], ptr[:])
                            it = g
                            po = ps_av.tile([128, 512], F32, tag='po',
                                            name=f'po{idx - 1}_{it}')
                            for t in range(NT // 2):
                                nc.tensor.matmul(
                                    po[:, 0:pbw],
                                    v8_t[:, 2 * t:2 * t + 2, it * 128:(it + 1) * 128],
                                    prev_at[:, 2 * t:2 * t + 2, :],
                                    start=(t == 0), stop=(t == NT // 2 - 1),
                                    perf_mode=DR)
                            nc.vector.tensor_mul(cur_w[:, it, :], po[:, 0:pbw],
                                                 prev_ut[:, it, :])

                    # u^T projection for THIS block (consumed next iteration).
                    if cur is not None:
                        for it in range(ITC):
                            pu = ps_s.tile([128, 512], F32, tag='pss',
                                           name=f'pu{idx}_{it}')
                            for hc in range(HC):
                                nc.tensor.matmul(pu[:, 0:bw],
                                                 Wu_t[:, hc, it * 128:(it + 1) * 128],
                                                 hsb[:, hc, :],
                                                 start=(hc == 0), stop=(hc == HC - 1))
                            nc.scalar.activation(cur_ut[:, it, :], pu[:, 0:bw],
                                                 AF.Silu)

                    # final: o[m-128, H] = sum_i w^T[:, m-tile].T @ Wo, scaled
                    if prev_at is not None:
                        for mt in range(pmt):
                            pf = [ps_f.tile([128, 512], F32, tag='psf',
                                            name=f'pf{idx - 1}_{mt}_{j}')
                                  for j in range(2)]
                            for it in range(ITC):
                                lhs = cur_w[:, it, mt * 128:(mt + 1) * 128]
                                for oc in range(2):
                                    nc.tensor.matmul(pf[oc][:], lhs,
                                                     Wo_t[:, it, oc * 512:(oc + 1) * 512],
                                                     start=(it == 0),
                                                     stop=(it == ITC - 1))
                            osb = finp.tile([128, H], BF16, tag='osb',
                                            name=f'osb{idx - 1}_{mt}')
                            for oc in range(2):
                                nc.scalar.activation(
                                    osb[:, oc * 512:(oc + 1) * 512], pf[oc][:],
                                    AF.Copy, bias=0.0, scale=rinv[:, mt:mt + 1])
                            row = pmoff + mt * 128
                            nc.sync.dma_start(out=o_out[row:row + 128, :],
                                              in_=osb[:])

                    prev_at = cur_at
                    prev_ut = cur_ut
                    prev = cur

    return nc


_CACHED = {}


def _prep_inputs(hidden_states, x_gcn, attention_mask, sin, cos, Wi, Wo, k_scale):
    bf = ml_dtypes.bfloat16
    f8 = ml_dtypes.float8_e4m3
    Wu = np.ascontiguousarray(Wi[:, :I]).astype(bf)
    Wz = Wi[:, 2 * I:]
    Wo_b = np.ascontiguousarray(Wo).astype(bf)
    Wv8 = (Wi[:, I:2 * I].astype(np.float32) * 16.0).astype(f8)
    Wz8_full = (np.concatenate([Wz[:, 0::2], Wz[:, 1::2]], axis=1)
                .astype(np.float32) * 16.0).astype(f8)

    sin2 = sin[0]          # [N, 64]
    cos2 = cos[0]
    kse, kso = k_scale[0::2], k_scale[1::2]
    TCc = np.concatenate([(cos2 * kse).T, (cos2 * kso).T], axis=0).astype(bf)
    TSc = np.concatenate([(sin2 * kso).T, (sin2 * kse).T], axis=0).astype(bf)

    # rotary(q) with softmax_plus scale folded in, per batch
    x1, x2 = x_gcn[..., 0::2], x_gcn[..., 1::2]
    c_, s_ = cos2[None], sin2[None]
    q_rot = np.concatenate([x1 * c_ - x2 * s_, x2 * c_ + x1 * s_], axis=-1)

    flats, flats8 = [], []
    for core in range(8):
        b, h = core // 2, core % 2
        l = float(attention_mask[b].sum())
        sc = np.log(l) / LOG512 / np.sqrt(DK)
        ebias = np.where(attention_mask[b] == 0, -30.0, 0.0)
        flat = np.empty(TOTAL, bf)
        flat[OFF_HS:OFF_HS + H * N] = np.ascontiguousarray(
            hidden_states[b].T).astype(bf).ravel()
        flat[OFF_QT:OFF_QT + DK * N] = np.ascontiguousarray(
            (q_rot[b] * sc).T).astype(bf).ravel()
        flat[OFF_TC:OFF_TC + DK * N] = TCc.ravel()
        flat[OFF_TS:OFF_TS + DK * N] = TSc.ravel()
        flat[OFF_EB:OFF_EB + 128 * NT] = np.ascontiguousarray(
            ebias.reshape(NT, 128).T).astype(bf).ravel()
        i0 = h * IH
        flat[OFF_WU:OFF_WU + H * IH] = np.ascontiguousarray(Wu[:, i0:i0 + IH]).ravel()
        flat[OFF_WO:OFF_WO + IH * H] = np.ascontiguousarray(Wo_b[i0:i0 + IH, :]).ravel()
        flats.append(flat)
        flat8 = np.empty(TOTAL8, f8)
        flat8[OFF8_WV:OFF8_WV + H * IH] = np.ascontiguousarray(
            Wv8[:, i0:i0 + IH]).ravel()
        flat8[OFF8_WZ:OFF8_WZ + H * DK] = Wz8_full.ravel()
        flats8.append(flat8)
    return flats, flats8


def _get_runner():
    if 'runner' in _CACHED:
        return _CACHED['runner']
    nc = build_program()
    bass2jax.install_neuronx_cc_hook()
    pn = nc.partition_id_tensor.name if nc.partition_id_tensor else None
    in_names, out_names, out_avals = [], [], []
    for alloc in nc.m.functions[0].allocations:
        if not isinstance(alloc, mybir.MemoryLocationSet):
            continue
        name = alloc.memorylocations[0].name
        if alloc.kind == 'ExternalInput':
            if name != pn:
                in_names.append(name)
        elif alloc.kind == 'ExternalOutput':
            out_names.append(name)
            shape = tuple(alloc.tensor_shape)
            dtype = mybir.dt.np(alloc.dtype)
            out_avals.append(jax.core.ShapedArray(shape, dtype))
    n_params = len(in_names)
    if pn is not None:
        in_names.append(pn)

    def _body(*args):
        ops = list(args)
        if pn is not None:
            ops.append(bass2jax.partition_id_tensor())
        return tuple(bass2jax._bass_exec_p.bind(
            *ops, out_avals=tuple(out_avals), in_names=tuple(in_names),
            out_names=tuple(out_names), lowering_input_output_aliases=(),
            sim_require_finite=True, sim_require_nnan=True, nc=nc))

    mesh = Mesh(np.asarray(jax.devices()[:8]), ('core',))
    sharding = NamedSharding(mesh, PartitionSpec('core'))
    sized = {'inp': (TOTAL, ml_dtypes.bfloat16),
             'inp8': (TOTAL8, ml_dtypes.float8_e4m3)}
    in_specs = tuple(
        jax.ShapeDtypeStruct((8 * sized[nm][0],), sized[nm][1],
                             sharding=sharding)
        for nm in in_names[:n_params])
    # AOT-compile with bass_effect suppressed so per-call dispatch takes
    # JAX's C++ fast path.
    sharded = bass2jax.fast_dispatch_compile(
        lambda: jax.jit(
            shard_map(_body, mesh=mesh,
                      in_specs=(PartitionSpec('core'),) * n_params,
                      out_specs=(PartitionSpec('core'),) * len(out_names),
                      check_rep=False),
            keep_unused=True).lower(*in_specs).compile())

    def put(arr):
        return jax.device_put(arr, sharding)

    _CACHED['runner'] = (nc, sharded, put, tuple(in_names[:n_params]))
    return _CACHED['runner']


def kernel(hidden_states, x_gcn, attention_mask, sin, cos, Wi, Wo, k_scale):
    _, sharded, put, in_names = _get_runner()
    flats, flats8 = _prep_inputs(np.asarray(hidden_states, np.float32),
                                 np.asarray(x_gcn, np.float32),
                                 np.asarray(attention_mask),
                                 np.asarray(sin, np.float32),
                                 np.asarray(cos, np.float32),
                                 np.asarray(Wi, np.float32),
                                 np.asarray(Wo, np.float32),
                                 np.asarray(k_scale, np.float32))
    cat = {'inp': np.concatenate(flats, axis=0),
           'inp8': np.concatenate(flats8, axis=0)}
    args = tuple(put(cat[nm]) for nm in in_names)
    res = np.asarray(sharded(*args)[0]).reshape(8, N, H).astype(np.float32)
    out = np.empty((B, N, H), np.float32)
    for b in range(B):
        out[b] = res[2 * b] + res[2 * b + 1]
    return out
